# revision 12
# baseline (speedup 1.0000x reference)
"""Trainium2 Bass kernel for nn_CrossSpaceAttention (batch 8, DIM=128, HEADS=8,
128x128 spatial). Data-parallel over batch: one sample per NeuronCore x8.

Per-core algorithm (all derived host-side constants folded):
  q = sum_t diag(qdw_t) @ qw @ shift_t(x0) + bias(h,w)      (dense 3x3 conv, 9 matmuls/tile on PE)
  k = likewise from x1
  G[c,d] = sum_n q[c,n] k[d,n] per head (via PE transposes + PE Gram accumulation)
  attn = softmax(0.25 * G / (|q_c| |k_d|))  per 32x32 head block
  y = sum_s (pw @ blockdiag(attn) @ diag(vdw_s) vw) @ shift_s(x2) + bias'  (attn+proj folded into conv)

Biases (including SAME-padding border effects) are applied as per-partition
tensor_scalar adds at PSUM-evacuation time: interior constant + edge rows/cols
+ corner fixups (exact).
"""
import numpy as np
import ml_dtypes

import concourse.bass as bass
import concourse.bacc as bacc
import concourse.mybir as mybir
import concourse.tile as tile
from concourse.bass_utils import run_bass_kernel_spmd
from concourse.masks import make_identity

BF = mybir.dt.bfloat16
F32 = mybir.dt.float32
BF_NP = ml_dtypes.bfloat16

C = 128          # input channels (DIM)
D2 = 256         # qkv channels
HH = 128         # spatial H
WW = 128         # spatial W
PH, PW = HH + 2, WW + 2
NTILE = 32       # spatial tiles of 4 rows x 128 cols (512 elements)
TAPS = [(dy, dx) for dy in (-1, 0, 1) for dx in (-1, 0, 1)]
ADD = mybir.AluOpType.add
MULT = mybir.AluOpType.mult
AF = mybir.ActivationFunctionType

_CACHE = {}


def _conv_block(nc, j, acc, wts, xp, extra=None):
    """9 accumulated tap matmuls into psum tile acc for spatial tile j.

    wts: SBUF (128, 9, M) lhsT per tap; xp: padded input (128, PH, PW)."""
    for t, (dy, dx) in enumerate(TAPS):
        rhs = xp[:, 4 * j + 1 + dy:4 * j + 5 + dy, 1 + dx:1 + dx + WW]
        lhsT = wts[:, t, :] if extra is None else wts[:, t, extra[0]:extra[1]]
        nc.tensor.matmul(acc[:, :, :], lhsT, rhs, start=(t == 0), stop=(t == 8))


def _bias_fixups(nc, st, cols, m, j, last_row=3):
    """Edge/corner bias adds on an evacuated tile st (128, 4, 128).

    cols: (128, n_chunks, 9) bias columns {int,dt,db,dl,dr,tl,tr,bl,br};
    interior (col 0) is applied during evacuation, not here."""
    cs = lambda i: cols[:, m, i:i + 1]
    nc.vector.tensor_scalar(out=st[:, :, 0:1], in0=st[:, :, 0:1],
                            scalar1=cs(3), scalar2=None, op0=ADD)
    nc.vector.tensor_scalar(out=st[:, :, 127:128], in0=st[:, :, 127:128],
                            scalar1=cs(4), scalar2=None, op0=ADD)
    if j == 0:
        nc.vector.tensor_scalar(out=st[:, 0, :], in0=st[:, 0, :],
                                scalar1=cs(1), scalar2=None, op0=ADD)
        nc.vector.tensor_scalar(out=st[:, 0, 0:1], in0=st[:, 0, 0:1],
                                scalar1=cs(5), scalar2=None, op0=ADD)
        nc.vector.tensor_scalar(out=st[:, 0, 127:128], in0=st[:, 0, 127:128],
                                scalar1=cs(6), scalar2=None, op0=ADD)
    if j == NTILE - 1:
        nc.vector.tensor_scalar(out=st[:, last_row, :], in0=st[:, last_row, :],
                                scalar1=cs(2), scalar2=None, op0=ADD)
        nc.vector.tensor_scalar(out=st[:, last_row, 0:1], in0=st[:, last_row, 0:1],
                                scalar1=cs(7), scalar2=None, op0=ADD)
        nc.vector.tensor_scalar(out=st[:, last_row, 127:128], in0=st[:, last_row, 127:128],
                                scalar1=cs(8), scalar2=None, op0=ADD)


def _load_pad(nc, xp, xd):
    """Zero the pad border of xp (128, PH, PW) and DMA the image into the interior."""
    nc.vector.memset(xp[:, 0, :], 0.0)
    nc.vector.memset(xp[:, PH - 1, :], 0.0)
    nc.vector.memset(xp[:, 1:PH - 1, 0:1], 0.0)
    nc.vector.memset(xp[:, 1:PH - 1, PW - 1:PW], 0.0)
    nc.sync.dma_start(out=xp[:, 1:PH - 1, 1:PW - 1], in_=xd[:, :, :])


def _build_nc():
    nc = bacc.Bacc(None, target_bir_lowering=False)

    x0d = nc.dram_tensor("x0", (C, HH, WW), BF, kind="ExternalInput")
    x1d = nc.dram_tensor("x1", (C, HH, WW), BF, kind="ExternalInput")
    x2d = nc.dram_tensor("x2", (C, HH, WW), BF, kind="ExternalInput")
    aqd = nc.dram_tensor("aq", (C, 9, D2), BF, kind="ExternalInput")
    akd = nc.dram_tensor("ak", (C, 9, D2), BF, kind="ExternalInput")
    qcd = nc.dram_tensor("qcols", (C, 2, 9), F32, kind="ExternalInput")
    kcd = nc.dram_tensor("kcols", (C, 2, 9), F32, kind="ExternalInput")
    cvd = nc.dram_tensor("cv", (C, 9, 2, C), F32, kind="ExternalInput")
    pwtd = nc.dram_tensor("pwT", (C, 2, C), F32, kind="ExternalInput")
    bvd = nc.dram_tensor("bv", (C, 2, 9), F32, kind="ExternalInput")
    pbd = nc.dram_tensor("pbrow", (1, C), F32, kind="ExternalInput")
    e0d = nc.dram_tensor("e0row", (1, 9), F32, kind="ExternalInput")
    onesd = nc.dram_tensor("ones1", (1, C), F32, kind="ExternalInput")
    yd = nc.dram_tensor("y", (C, HH, WW), F32, kind="ExternalOutput")
    import os
    dbg = bool(os.environ.get("KDEBUG"))
    if dbg:
        gdumpd = nc.dram_tensor("gdump", (128, 2, 128), F32, kind="ExternalOutput")
        adumpd = nc.dram_tensor("adump", (128, 2, D2), F32, kind="ExternalOutput")
        ndumpd = nc.dram_tensor("ndump", (128, 4), F32, kind="ExternalOutput")
        edumpd = nc.dram_tensor("edump", (128, 9, C), BF, kind="ExternalOutput")
        cdumpd = nc.dram_tensor("cdump", (128, 9), F32, kind="ExternalOutput")
        qtdumpd = nc.dram_tensor("qtdump", (128, 128, D2), BF, kind="ExternalOutput")

    with tile.TileContext(nc) as tc:
        with (
            tc.tile_pool(name="consts", bufs=1) as consts,
            tc.tile_pool(name="xpad", bufs=2) as xpad,
            tc.tile_pool(name="qtp", bufs=1) as qtp,
            tc.tile_pool(name="ktile", bufs=4) as ktile,
            tc.tile_pool(name="stage", bufs=4) as stage,
            tc.tile_pool(name="sqscr", bufs=2) as sqscr,
            tc.tile_pool(name="small", bufs=1) as small,
            tc.tile_pool(name="ysb", bufs=4) as ysb,
            tc.tile_pool(name="cpsum", bufs=3, space="PSUM") as cpsum,
            tc.tile_pool(name="tpsum", bufs=2, space="PSUM") as tpsum,
            tc.tile_pool(name="gpsum", bufs=1, space="PSUM") as gpsum,
            tc.tile_pool(name="mpsum", bufs=1, space="PSUM") as mpsum,
        ):
            # ---- constants ----
            aq = consts.tile([C, 9, D2], BF)
            nc.sync.dma_start(out=aq, in_=aqd[:, :, :])
            ak = consts.tile([C, 9, D2], BF)
            nc.sync.dma_start(out=ak, in_=akd[:, :, :])
            qcols = consts.tile([C, 2, 9], F32)
            nc.sync.dma_start(out=qcols, in_=qcd[:, :, :])
            kcols = consts.tile([C, 2, 9], F32)
            nc.sync.dma_start(out=kcols, in_=kcd[:, :, :])
            cv = consts.tile([C, 9, 2, C], F32)
            nc.sync.dma_start(out=cv, in_=cvd[:, :, :, :])
            pwt = consts.tile([C, 2, C], F32)
            nc.sync.dma_start(out=pwt, in_=pwtd[:, :, :])
            bv = consts.tile([C, 2, 9], F32)
            nc.sync.dma_start(out=bv, in_=bvd[:, :, :])
            pbrow = consts.tile([1, C], F32)
            nc.sync.dma_start(out=pbrow, in_=pbd[:, :])
            e0row = consts.tile([1, 9], F32)
            nc.sync.dma_start(out=e0row, in_=e0d[:, :])
            ones1 = consts.tile([1, C], F32)
            nc.sync.dma_start(out=ones1, in_=onesd[:, :])
            identb = consts.tile([128, 128], BF)
            make_identity(nc, identb)
            identf = consts.tile([128, 128], F32)
            make_identity(nc, identf)

            # ---- accumulators / attn-stage tiles ----
            qT = qtp.tile([128, 128, D2], BF)     # [n_in_chunk, n_chunk, c]
            qn2 = small.tile([128, 2, NTILE], F32)
            kn2 = small.tile([128, 2, NTILE], F32)
            qinv = small.tile([128, 2], F32)
            kinv = small.tile([128, 2], F32)
            kir = small.tile([1, 2, C], F32)
            kb = small.tile([128, 2, C], F32)
            lblk = small.tile([128, 2, 32], F32)
            ablk = small.tile([128, 2, 32], F32)
            rs = small.tile([128, 2], F32)
            rr = small.tile([128, 2], F32)
            attnBD = small.tile([128, 2, D2], F32)
            pat = small.tile([128, 2, C], F32)
            eall = small.tile([128, 9, C], BF)
            coly = small.tile([128, 9], F32)

            x0p = xpad.tile([C, PH, PW], BF, tag="xp")
            _load_pad(nc, x0p, x0d)
            x1p = xpad.tile([C, PH, PW], BF, tag="xp")
            _load_pad(nc, x1p, x1d)

            nc.vector.memset(attnBD.rearrange("p a b -> p (a b)"), 0.0)

            # ---- q / k convs, staging, transposes, norms, gram ----
            for conv in ("q", "k"):
                wts, xp, cols, n2 = ((aq, x0p, qcols, qn2) if conv == "q"
                                     else (ak, x1p, kcols, kn2))
                for j in range(NTILE):
                    if conv == "k":
                        ktt = ktile.tile([128, 4, D2], BF)
                    for m in range(2):
                        acc = cpsum.tile([128, 4, 128], F32)
                        _conv_block(nc, j, acc, wts, xp, extra=(128 * m, 128 * m + 128))
                        st = stage.tile([128, 4, 128], BF)
                        nc.vector.tensor_scalar(out=st, in0=acc,
                                                scalar1=cols[:, m, 0:1],
                                                scalar2=None, op0=ADD)
                        _bias_fixups(nc, st, cols, m, j)
                        sq = sqscr.tile([128, 512], BF)
                        nc.scalar.activation(out=sq, in_=st.rearrange("p a b -> p (a b)"),
                                             func=AF.Square,
                                             accum_out=n2[:, m, j:j + 1])
                        tp = tpsum.tile([128, 4, 128], BF)
                        for b in range(4):
                            nc.tensor.transpose(tp[:, b, :], st[:, b, :], identb)
                        if conv == "q":
                            nc.scalar.copy(
                                qT[:, 4 * j:4 * j + 4, 128 * m:128 * m + 128],
                                tp[:, :, :])
                        else:
                            nc.scalar.copy(
                                ktt[:, :, 128 * m:128 * m + 128], tp[:, :, :])
                    if conv == "k":
                        if j == 0:
                            G0 = gpsum.tile([128, 128], F32, tag="G0")
                            G1 = gpsum.tile([128, 128], F32, tag="G1")
                        for b in range(4):
                            jn = 4 * j + b
                            for g, Gt in ((0, G0), (1, G1)):
                                nc.tensor.matmul(
                                    Gt[:, :],
                                    qT[:, jn, 128 * g:128 * g + 128],
                                    ktt[:, b, 128 * g:128 * g + 128],
                                    start=(jn == 0), stop=(jn == 4 * NTILE - 1))

            x2p = xpad.tile([C, PH, PW], BF, tag="xp")
            _load_pad(nc, x2p, x2d)

            # ---- norms -> qinv = 1/|q|, kinv = 0.25/|k| ----
            for m in range(2):
                nc.vector.tensor_reduce(out=qinv[:, m:m + 1], in_=qn2[:, m, :],
                                        axis=mybir.AxisListType.X, op=ADD)
                nc.vector.tensor_reduce(out=kinv[:, m:m + 1], in_=kn2[:, m, :],
                                        axis=mybir.AxisListType.X, op=ADD)
            nc.scalar.activation(out=qinv, in_=qinv, func=AF.Sqrt)
            nc.scalar.activation(out=kinv, in_=kinv, func=AF.Sqrt, scale=16.0)
            nc.vector.reciprocal(out=qinv, in_=qinv)
            nc.vector.reciprocal(out=kinv, in_=kinv)

            # broadcast kinv across partitions: kb[p, g, d] = kinv[d, g]
            for g in range(2):
                kt = mpsum.tile([1, C], F32, tag="mp")
                nc.tensor.transpose(kt, kinv[:, g:g + 1], identf)
                nc.vector.tensor_copy(kir[:, g, :], kt)
                kbp = mpsum.tile([128, C], F32, tag="mp")
                nc.tensor.matmul(kbp, ones1, kir[:, g, :], start=True, stop=True)
                nc.vector.tensor_copy(kb[:, g, :], kbp)

            # ---- softmax per 32x32 head block -> attnBD ----
            for g in range(2):
                for b in range(4):
                    p0 = 32 * b
                    Gt = G0 if g == 0 else G1
                    nc.vector.tensor_tensor(
                        out=lblk[p0:p0 + 32, g, :],
                        in0=Gt[p0:p0 + 32, p0:p0 + 32],
                        in1=kb[p0:p0 + 32, g, p0:p0 + 32],
                        op=MULT)
                    nc.scalar.activation(
                        out=ablk[p0:p0 + 32, g, :], in_=lblk[p0:p0 + 32, g, :],
                        func=AF.Exp, scale=qinv[p0:p0 + 32, g:g + 1],
                        accum_out=rs[p0:p0 + 32, g:g + 1])
                nc.vector.reciprocal(out=rr[:, g:g + 1], in_=rs[:, g:g + 1])
                for b in range(4):
                    p0 = 32 * b
                    nc.vector.tensor_scalar(
                        out=attnBD[p0:p0 + 32, g, 128 * g + p0:128 * g + p0 + 32],
                        in0=ablk[p0:p0 + 32, g, :],
                        scalar1=rr[p0:p0 + 32, g:g + 1], scalar2=None, op0=MULT)

            if dbg:
                gsb = small.tile([128, 2, 128], F32)
                nc.vector.tensor_copy(gsb[:, 0, :], G0)
                nc.vector.tensor_copy(gsb[:, 1, :], G1)
                nc.sync.dma_start(out=gdumpd[:, :, :], in_=gsb)
                nc.sync.dma_start(out=adumpd[:, :, :], in_=attnBD)
                nvd = small.tile([128, 4], F32)
                nc.vector.tensor_copy(nvd[:, 0:2], qinv)
                nc.vector.tensor_copy(nvd[:, 2:4], kinv)
                nc.sync.dma_start(out=ndumpd[:, :], in_=nvd)
                nc.sync.dma_start(out=qtdumpd[:, :, :], in_=qT)

            # ---- PA^T = attnBD^T @ pw^T ----
            patp = mpsum.tile([128, 2, C], F32, tag="mp")
            for mc in range(2):
                for kc in range(2):
                    nc.tensor.matmul(patp[:, mc, :],
                                     attnBD[:, kc, 128 * mc:128 * mc + 128],
                                     pwt[:, kc, :], start=(kc == 0), stop=(kc == 1))
            nc.vector.tensor_copy(pat.rearrange("p a b -> p (a b)"),
                                  patp.rearrange("p a b -> p (a b)"))

            # ---- E_s^T = C_s^T @ PA^T  (y-conv weights), and bias columns ----
            for s in range(9):
                ep = mpsum.tile([128, C], F32, tag="mp")
                for kc in range(2):
                    nc.tensor.matmul(ep, cv[:, s, kc, :], pat[:, kc, :],
                                     start=(kc == 0), stop=(kc == 1))
                nc.vector.tensor_copy(eall[:, s, :], ep)
            wp = mpsum.tile([128, 9], F32, tag="mp")
            for kc in range(2):
                nc.tensor.matmul(wp, pat[:, kc, :], bv[:, kc, :],
                                 start=(kc == 0), stop=False)
            nc.tensor.matmul(wp, pbrow, e0row, start=False, stop=True)
            nc.vector.tensor_copy(coly, wp)
            if dbg:
                nc.sync.dma_start(out=edumpd[:, :, :], in_=eall)
                nc.sync.dma_start(out=cdumpd[:, :], in_=coly)

            # ---- y conv ----
            for j in range(NTILE):
                acc = cpsum.tile([128, 4, 128], F32)
                _conv_block(nc, j, acc, eall, x2p)
                yt = ysb.tile([128, 4, 128], F32)
                nc.vector.tensor_scalar(out=yt, in0=acc, scalar1=coly[:, 0:1],
                                        scalar2=None, op0=ADD)
                _bias_fixups(nc, yt, coly.rearrange("p (a b) -> p a b", a=1), 0, j)
                nc.sync.dma_start(out=yd[:, 4 * j:4 * j + 4, :], in_=yt)

    nc.compile()
    return nc


def _host_consts(qw, qb, kw, kb, vw, vb, qdw, qdb, kdw, kdb, vdw, vdb, pw, pb):
    """Fold all static weights into the forms the kernel consumes."""
    qw2, kw2, vw2, pw2 = [w[:, :, 0, 0].astype(np.float64) for w in (qw, kw, vw, pw)]
    qd, kd, vd = [w[:, 0].astype(np.float64) for w in (qdw, kdw, vdw)]

    def conv_w(d, w2):
        # (C, 9, D2): lhsT per tap: A_t^T where A_t = diag(d_t) @ w2
        a = np.stack([(d[:, dy + 1, dx + 1][:, None] * w2).T
                      for (dy, dx) in TAPS], axis=1)
        return a.astype(np.float32).astype(BF_NP)

    def bias_cols(b1, db, d):
        cols = np.stack([
            db + b1 * d.sum((-2, -1)),
            -b1 * d[:, 0, :].sum(-1), -b1 * d[:, 2, :].sum(-1),
            -b1 * d[:, :, 0].sum(-1), -b1 * d[:, :, 2].sum(-1),
            b1 * d[:, 0, 0], b1 * d[:, 0, 2], b1 * d[:, 2, 0], b1 * d[:, 2, 2],
        ], axis=-1)  # (256, 9)
        return cols.reshape(2, 128, 9).transpose(1, 0, 2).astype(np.float32)

    cv = np.stack([(vd[:, dy + 1, dx + 1][:, None] * vw2)
                   for (dy, dx) in TAPS])             # (9, 256, 128)
    cv = cv.reshape(9, 2, 128, 128).transpose(2, 0, 1, 3).astype(np.float32)
    pwT = pw2.T.reshape(2, 128, 128).transpose(1, 0, 2).astype(np.float32)
    e0 = np.zeros((1, 9), np.float32)
    e0[0, 0] = 1.0
    return {
        "aq": conv_w(qd, qw2), "ak": conv_w(kd, kw2),
        "qcols": bias_cols(qb.astype(np.float64), qdb.astype(np.float64), qd),
        "kcols": bias_cols(kb.astype(np.float64), kdb.astype(np.float64), kd),
        "cv": cv, "pwT": pwT,
        "bv": bias_cols(vb.astype(np.float64), vdb.astype(np.float64), vd),
        "pbrow": pb.reshape(1, C).astype(np.float32),
        "e0row": e0,
        "ones1": np.ones((1, C), np.float32),
    }


def kernel(**inputs):
    if "nc" not in _CACHE:
        _CACHE["nc"] = _build_nc()
    nc = _CACHE["nc"]

    consts = _host_consts(**{k: np.asarray(inputs[k]) for k in
                             ("qw", "qb", "kw", "kb", "vw", "vb", "qdw", "qdb",
                              "kdw", "kdb", "vdw", "vdb", "pw", "pb")})
    x0 = np.asarray(inputs["x0"]).astype(BF_NP)
    x1 = np.asarray(inputs["x1"]).astype(BF_NP)
    x2 = np.asarray(inputs["x2"]).astype(BF_NP)
    n_cores = x0.shape[0]
    in_maps = [dict(consts, x0=x0[i], x1=x1[i], x2=x2[i]) for i in range(n_cores)]
    res = run_bass_kernel_spmd(nc, in_maps, list(range(n_cores)))
    _CACHE["last_res"] = res
    return np.stack([np.asarray(r["y"]) for r in res.results]).astype(np.float32)


def kernel_sim(**inputs):
    """CoreSim validation path: run sample 0 only through the simulator."""
    from concourse.bass_interp import CoreSim

    if "nc" not in _CACHE:
        _CACHE["nc"] = _build_nc()
    nc = _CACHE["nc"]
    consts = _host_consts(**{k: np.asarray(inputs[k]) for k in
                             ("qw", "qb", "kw", "kb", "vw", "vb", "qdw", "qdb",
                              "kdw", "kdb", "vdw", "vdb", "pw", "pb")})
    sim = CoreSim(nc)
    for name, arr in consts.items():
        sim.tensor(name)[:] = arr
    for name in ("x0", "x1", "x2"):
        sim.tensor(name)[:] = np.asarray(inputs[name])[0].astype(BF_NP)
    sim.simulate()
    return np.array(sim.tensor("y"))[None].astype(np.float32)


# revision 13
# speedup vs baseline: 1.0005x; 1.0005x over previous
"""Trainium2 Bass kernel for nn_CrossSpaceAttention (batch 8, DIM=128, HEADS=8,
128x128 spatial). Data-parallel over batch: one sample per NeuronCore x8.

Per-core algorithm (all derived host-side constants folded):
  q = sum_t diag(qdw_t) @ qw @ shift_t(x0) + bias(h,w)      (dense 3x3 conv, 9 matmuls/tile on PE)
  k = likewise from x1
  G[c,d] = sum_n q[c,n] k[d,n] per head (via PE transposes + PE Gram accumulation)
  attn = softmax(0.25 * G / (|q_c| |k_d|))  per 32x32 head block
  y = sum_s (pw @ blockdiag(attn) @ diag(vdw_s) vw) @ shift_s(x2) + bias'  (attn+proj folded into conv)

Biases (including SAME-padding border effects) are applied as per-partition
tensor_scalar adds at PSUM-evacuation time: interior constant + edge rows/cols
+ corner fixups (exact).
"""
import numpy as np
import ml_dtypes

import concourse.bass as bass
import concourse.bacc as bacc
import concourse.mybir as mybir
import concourse.tile as tile
from concourse.bass_utils import run_bass_kernel_spmd
from concourse.masks import make_identity

BF = mybir.dt.bfloat16
F32 = mybir.dt.float32
BF_NP = ml_dtypes.bfloat16

C = 128          # input channels (DIM)
D2 = 256         # qkv channels
HH = 128         # spatial H
WW = 128         # spatial W
PH, PW = HH + 2, WW + 2
NTILE = 32       # spatial tiles of 4 rows x 128 cols (512 elements)
TAPS = [(dy, dx) for dy in (-1, 0, 1) for dx in (-1, 0, 1)]
ADD = mybir.AluOpType.add
MULT = mybir.AluOpType.mult
AF = mybir.ActivationFunctionType

_CACHE = {}


def _conv_block(nc, j, acc, wts, xp, extra=None):
    """9 accumulated tap matmuls into psum tile acc for spatial tile j.

    wts: SBUF (128, 9, M) lhsT per tap; xp: padded input (128, PH, PW)."""
    for t, (dy, dx) in enumerate(TAPS):
        rhs = xp[:, 4 * j + 1 + dy:4 * j + 5 + dy, 1 + dx:1 + dx + WW]
        lhsT = wts[:, t, :] if extra is None else wts[:, t, extra[0]:extra[1]]
        nc.tensor.matmul(acc[:, :, :], lhsT, rhs, start=(t == 0), stop=(t == 8))


def _bias_fixups(nc, st, cols, m, j, last_row=3):
    """Edge/corner bias adds on an evacuated tile st (128, 4, 128).

    cols: (128, n_chunks, 9) bias columns {int,dt,db,dl,dr,tl,tr,bl,br};
    interior (col 0) is applied during evacuation, not here."""
    cs = lambda i: cols[:, m, i:i + 1]
    nc.vector.tensor_scalar(out=st[:, :, 0:1], in0=st[:, :, 0:1],
                            scalar1=cs(3), scalar2=None, op0=ADD)
    nc.vector.tensor_scalar(out=st[:, :, 127:128], in0=st[:, :, 127:128],
                            scalar1=cs(4), scalar2=None, op0=ADD)
    if j == 0:
        nc.vector.tensor_scalar(out=st[:, 0, :], in0=st[:, 0, :],
                                scalar1=cs(1), scalar2=None, op0=ADD)
        nc.vector.tensor_scalar(out=st[:, 0, 0:1], in0=st[:, 0, 0:1],
                                scalar1=cs(5), scalar2=None, op0=ADD)
        nc.vector.tensor_scalar(out=st[:, 0, 127:128], in0=st[:, 0, 127:128],
                                scalar1=cs(6), scalar2=None, op0=ADD)
    if j == NTILE - 1:
        nc.vector.tensor_scalar(out=st[:, last_row, :], in0=st[:, last_row, :],
                                scalar1=cs(2), scalar2=None, op0=ADD)
        nc.vector.tensor_scalar(out=st[:, last_row, 0:1], in0=st[:, last_row, 0:1],
                                scalar1=cs(7), scalar2=None, op0=ADD)
        nc.vector.tensor_scalar(out=st[:, last_row, 127:128], in0=st[:, last_row, 127:128],
                                scalar1=cs(8), scalar2=None, op0=ADD)


def _load_pad(nc, xp, xd):
    """Zero the pad border of xp (128, PH, PW) and DMA the image into the interior."""
    nc.vector.memset(xp[:, 0, :], 0.0)
    nc.vector.memset(xp[:, PH - 1, :], 0.0)
    nc.vector.memset(xp[:, 1:PH - 1, 0:1], 0.0)
    nc.vector.memset(xp[:, 1:PH - 1, PW - 1:PW], 0.0)
    nc.sync.dma_start(out=xp[:, 1:PH - 1, 1:PW - 1], in_=xd[:, :, :])


def _build_nc():
    nc = bacc.Bacc(None, target_bir_lowering=False)

    x0d = nc.dram_tensor("x0", (C, HH, WW), BF, kind="ExternalInput")
    x1d = nc.dram_tensor("x1", (C, HH, WW), BF, kind="ExternalInput")
    x2d = nc.dram_tensor("x2", (C, HH, WW), BF, kind="ExternalInput")
    aqd = nc.dram_tensor("aq", (C, 9, D2), BF, kind="ExternalInput")
    akd = nc.dram_tensor("ak", (C, 9, D2), BF, kind="ExternalInput")
    qcd = nc.dram_tensor("qcols", (C, 2, 9), F32, kind="ExternalInput")
    kcd = nc.dram_tensor("kcols", (C, 2, 9), F32, kind="ExternalInput")
    cvd = nc.dram_tensor("cv", (C, 9, 2, C), F32, kind="ExternalInput")
    pwtd = nc.dram_tensor("pwT", (C, 2, C), F32, kind="ExternalInput")
    bvd = nc.dram_tensor("bv", (C, 2, 9), F32, kind="ExternalInput")
    pbd = nc.dram_tensor("pbrow", (1, C), F32, kind="ExternalInput")
    e0d = nc.dram_tensor("e0row", (1, 9), F32, kind="ExternalInput")
    onesd = nc.dram_tensor("ones1", (1, C), F32, kind="ExternalInput")
    yd = nc.dram_tensor("y", (C, HH, WW), F32, kind="ExternalOutput")
    import os
    dbg = bool(os.environ.get("KDEBUG"))
    if dbg:
        gdumpd = nc.dram_tensor("gdump", (128, 2, 128), F32, kind="ExternalOutput")
        adumpd = nc.dram_tensor("adump", (128, 2, D2), F32, kind="ExternalOutput")
        ndumpd = nc.dram_tensor("ndump", (128, 4), F32, kind="ExternalOutput")
        edumpd = nc.dram_tensor("edump", (128, 9, C), BF, kind="ExternalOutput")
        cdumpd = nc.dram_tensor("cdump", (128, 9), F32, kind="ExternalOutput")
        qtdumpd = nc.dram_tensor("qtdump", (128, 128, D2), BF, kind="ExternalOutput")

    with tile.TileContext(nc) as tc:
        with (
            tc.tile_pool(name="consts", bufs=1) as consts,
            tc.tile_pool(name="xpad", bufs=2) as xpad,
            tc.tile_pool(name="qtp", bufs=1) as qtp,
            tc.tile_pool(name="ktile", bufs=6) as ktile,
            tc.tile_pool(name="stage", bufs=6) as stage,
            tc.tile_pool(name="sqscr", bufs=4) as sqscr,
            tc.tile_pool(name="small", bufs=1) as small,
            tc.tile_pool(name="ysb", bufs=6) as ysb,
            tc.tile_pool(name="cpsum", bufs=3, space="PSUM") as cpsum,
            tc.tile_pool(name="tpsum", bufs=2, space="PSUM") as tpsum,
            tc.tile_pool(name="gpsum", bufs=1, space="PSUM") as gpsum,
            tc.tile_pool(name="mpsum", bufs=1, space="PSUM") as mpsum,
        ):
            # ---- constants ----
            aq = consts.tile([C, 9, D2], BF)
            nc.sync.dma_start(out=aq, in_=aqd[:, :, :])
            ak = consts.tile([C, 9, D2], BF)
            nc.sync.dma_start(out=ak, in_=akd[:, :, :])
            qcols = consts.tile([C, 2, 9], F32)
            nc.sync.dma_start(out=qcols, in_=qcd[:, :, :])
            kcols = consts.tile([C, 2, 9], F32)
            nc.sync.dma_start(out=kcols, in_=kcd[:, :, :])
            cv = consts.tile([C, 9, 2, C], F32)
            nc.sync.dma_start(out=cv, in_=cvd[:, :, :, :])
            pwt = consts.tile([C, 2, C], F32)
            nc.sync.dma_start(out=pwt, in_=pwtd[:, :, :])
            bv = consts.tile([C, 2, 9], F32)
            nc.sync.dma_start(out=bv, in_=bvd[:, :, :])
            pbrow = consts.tile([1, C], F32)
            nc.sync.dma_start(out=pbrow, in_=pbd[:, :])
            e0row = consts.tile([1, 9], F32)
            nc.sync.dma_start(out=e0row, in_=e0d[:, :])
            ones1 = consts.tile([1, C], F32)
            nc.sync.dma_start(out=ones1, in_=onesd[:, :])
            identb = consts.tile([128, 128], BF)
            make_identity(nc, identb)
            identf = consts.tile([128, 128], F32)
            make_identity(nc, identf)

            # ---- accumulators / attn-stage tiles ----
            qT = qtp.tile([128, 128, D2], BF)     # [n_in_chunk, n_chunk, c]
            qn2 = small.tile([128, 2, NTILE], F32)
            kn2 = small.tile([128, 2, NTILE], F32)
            qinv = small.tile([128, 2], F32)
            kinv = small.tile([128, 2], F32)
            kir = small.tile([1, 2, C], F32)
            kb = small.tile([128, 2, C], F32)
            lblk = small.tile([128, 2, 32], F32)
            ablk = small.tile([128, 2, 32], F32)
            rs = small.tile([128, 2], F32)
            rr = small.tile([128, 2], F32)
            attnBD = small.tile([128, 2, D2], F32)
            pat = small.tile([128, 2, C], F32)
            eall = small.tile([128, 9, C], BF)
            coly = small.tile([128, 9], F32)

            x0p = xpad.tile([C, PH, PW], BF, tag="xp")
            _load_pad(nc, x0p, x0d)
            x1p = xpad.tile([C, PH, PW], BF, tag="xp")
            _load_pad(nc, x1p, x1d)

            nc.vector.memset(attnBD.rearrange("p a b -> p (a b)"), 0.0)

            # ---- q / k convs, staging, transposes, norms, gram ----
            for conv in ("q", "k"):
                wts, xp, cols, n2 = ((aq, x0p, qcols, qn2) if conv == "q"
                                     else (ak, x1p, kcols, kn2))
                for j in range(NTILE):
                    if conv == "k":
                        ktt = ktile.tile([128, 4, D2], BF)
                    for m in range(2):
                        acc = cpsum.tile([128, 4, 128], F32)
                        _conv_block(nc, j, acc, wts, xp, extra=(128 * m, 128 * m + 128))
                        st = stage.tile([128, 4, 128], BF)
                        nc.vector.tensor_scalar(out=st, in0=acc,
                                                scalar1=cols[:, m, 0:1],
                                                scalar2=None, op0=ADD)
                        _bias_fixups(nc, st, cols, m, j)
                        sq = sqscr.tile([128, 512], BF)
                        nc.scalar.activation(out=sq, in_=st.rearrange("p a b -> p (a b)"),
                                             func=AF.Square,
                                             accum_out=n2[:, m, j:j + 1])
                        tp = tpsum.tile([128, 4, 128], BF)
                        for b in range(4):
                            nc.tensor.transpose(tp[:, b, :], st[:, b, :], identb)
                        if conv == "q":
                            nc.scalar.copy(
                                qT[:, 4 * j:4 * j + 4, 128 * m:128 * m + 128],
                                tp[:, :, :])
                        else:
                            nc.scalar.copy(
                                ktt[:, :, 128 * m:128 * m + 128], tp[:, :, :])
                    if conv == "k":
                        if j == 0:
                            G0 = gpsum.tile([128, 128], F32, tag="G0")
                            G1 = gpsum.tile([128, 128], F32, tag="G1")
                        for b in range(4):
                            jn = 4 * j + b
                            for g, Gt in ((0, G0), (1, G1)):
                                nc.tensor.matmul(
                                    Gt[:, :],
                                    qT[:, jn, 128 * g:128 * g + 128],
                                    ktt[:, b, 128 * g:128 * g + 128],
                                    start=(jn == 0), stop=(jn == 4 * NTILE - 1))

            x2p = xpad.tile([C, PH, PW], BF, tag="xp")
            _load_pad(nc, x2p, x2d)

            # ---- norms -> qinv = 1/|q|, kinv = 0.25/|k| ----
            for m in range(2):
                nc.vector.tensor_reduce(out=qinv[:, m:m + 1], in_=qn2[:, m, :],
                                        axis=mybir.AxisListType.X, op=ADD)
                nc.vector.tensor_reduce(out=kinv[:, m:m + 1], in_=kn2[:, m, :],
                                        axis=mybir.AxisListType.X, op=ADD)
            nc.scalar.activation(out=qinv, in_=qinv, func=AF.Sqrt)
            nc.scalar.activation(out=kinv, in_=kinv, func=AF.Sqrt, scale=16.0)
            nc.vector.reciprocal(out=qinv, in_=qinv)
            nc.vector.reciprocal(out=kinv, in_=kinv)

            # broadcast kinv across partitions: kb[p, g, d] = kinv[d, g]
            for g in range(2):
                kt = mpsum.tile([1, C], F32, tag="mp")
                nc.tensor.transpose(kt, kinv[:, g:g + 1], identf)
                nc.vector.tensor_copy(kir[:, g, :], kt)
                kbp = mpsum.tile([128, C], F32, tag="mp")
                nc.tensor.matmul(kbp, ones1, kir[:, g, :], start=True, stop=True)
                nc.vector.tensor_copy(kb[:, g, :], kbp)

            # ---- softmax per 32x32 head block -> attnBD ----
            for g in range(2):
                for b in range(4):
                    p0 = 32 * b
                    Gt = G0 if g == 0 else G1
                    nc.vector.tensor_tensor(
                        out=lblk[p0:p0 + 32, g, :],
                        in0=Gt[p0:p0 + 32, p0:p0 + 32],
                        in1=kb[p0:p0 + 32, g, p0:p0 + 32],
                        op=MULT)
                    nc.scalar.activation(
                        out=ablk[p0:p0 + 32, g, :], in_=lblk[p0:p0 + 32, g, :],
                        func=AF.Exp, scale=qinv[p0:p0 + 32, g:g + 1],
                        accum_out=rs[p0:p0 + 32, g:g + 1])
                nc.vector.reciprocal(out=rr[:, g:g + 1], in_=rs[:, g:g + 1])
                for b in range(4):
                    p0 = 32 * b
                    nc.vector.tensor_scalar(
                        out=attnBD[p0:p0 + 32, g, 128 * g + p0:128 * g + p0 + 32],
                        in0=ablk[p0:p0 + 32, g, :],
                        scalar1=rr[p0:p0 + 32, g:g + 1], scalar2=None, op0=MULT)

            if dbg:
                gsb = small.tile([128, 2, 128], F32)
                nc.vector.tensor_copy(gsb[:, 0, :], G0)
                nc.vector.tensor_copy(gsb[:, 1, :], G1)
                nc.sync.dma_start(out=gdumpd[:, :, :], in_=gsb)
                nc.sync.dma_start(out=adumpd[:, :, :], in_=attnBD)
                nvd = small.tile([128, 4], F32)
                nc.vector.tensor_copy(nvd[:, 0:2], qinv)
                nc.vector.tensor_copy(nvd[:, 2:4], kinv)
                nc.sync.dma_start(out=ndumpd[:, :], in_=nvd)
                nc.sync.dma_start(out=qtdumpd[:, :, :], in_=qT)

            # ---- PA^T = attnBD^T @ pw^T ----
            patp = mpsum.tile([128, 2, C], F32, tag="mp")
            for mc in range(2):
                for kc in range(2):
                    nc.tensor.matmul(patp[:, mc, :],
                                     attnBD[:, kc, 128 * mc:128 * mc + 128],
                                     pwt[:, kc, :], start=(kc == 0), stop=(kc == 1))
            nc.vector.tensor_copy(pat.rearrange("p a b -> p (a b)"),
                                  patp.rearrange("p a b -> p (a b)"))

            # ---- E_s^T = C_s^T @ PA^T  (y-conv weights), and bias columns ----
            for s in range(9):
                ep = mpsum.tile([128, C], F32, tag="mp")
                for kc in range(2):
                    nc.tensor.matmul(ep, cv[:, s, kc, :], pat[:, kc, :],
                                     start=(kc == 0), stop=(kc == 1))
                nc.vector.tensor_copy(eall[:, s, :], ep)
            wp = mpsum.tile([128, 9], F32, tag="mp")
            for kc in range(2):
                nc.tensor.matmul(wp, pat[:, kc, :], bv[:, kc, :],
                                 start=(kc == 0), stop=False)
            nc.tensor.matmul(wp, pbrow, e0row, start=False, stop=True)
            nc.vector.tensor_copy(coly, wp)
            if dbg:
                nc.sync.dma_start(out=edumpd[:, :, :], in_=eall)
                nc.sync.dma_start(out=cdumpd[:, :], in_=coly)

            # ---- y conv ----
            for j in range(NTILE):
                acc = cpsum.tile([128, 4, 128], F32)
                _conv_block(nc, j, acc, eall, x2p)
                yt = ysb.tile([128, 4, 128], F32)
                nc.vector.tensor_scalar(out=yt, in0=acc, scalar1=coly[:, 0:1],
                                        scalar2=None, op0=ADD)
                _bias_fixups(nc, yt, coly.rearrange("p (a b) -> p a b", a=1), 0, j)
                nc.sync.dma_start(out=yd[:, 4 * j:4 * j + 4, :], in_=yt)

    nc.compile()
    return nc


def _host_consts(qw, qb, kw, kb, vw, vb, qdw, qdb, kdw, kdb, vdw, vdb, pw, pb):
    """Fold all static weights into the forms the kernel consumes."""
    qw2, kw2, vw2, pw2 = [w[:, :, 0, 0].astype(np.float64) for w in (qw, kw, vw, pw)]
    qd, kd, vd = [w[:, 0].astype(np.float64) for w in (qdw, kdw, vdw)]

    def conv_w(d, w2):
        # (C, 9, D2): lhsT per tap: A_t^T where A_t = diag(d_t) @ w2
        a = np.stack([(d[:, dy + 1, dx + 1][:, None] * w2).T
                      for (dy, dx) in TAPS], axis=1)
        return a.astype(np.float32).astype(BF_NP)

    def bias_cols(b1, db, d):
        cols = np.stack([
            db + b1 * d.sum((-2, -1)),
            -b1 * d[:, 0, :].sum(-1), -b1 * d[:, 2, :].sum(-1),
            -b1 * d[:, :, 0].sum(-1), -b1 * d[:, :, 2].sum(-1),
            b1 * d[:, 0, 0], b1 * d[:, 0, 2], b1 * d[:, 2, 0], b1 * d[:, 2, 2],
        ], axis=-1)  # (256, 9)
        return cols.reshape(2, 128, 9).transpose(1, 0, 2).astype(np.float32)

    cv = np.stack([(vd[:, dy + 1, dx + 1][:, None] * vw2)
                   for (dy, dx) in TAPS])             # (9, 256, 128)
    cv = cv.reshape(9, 2, 128, 128).transpose(2, 0, 1, 3).astype(np.float32)
    pwT = pw2.T.reshape(2, 128, 128).transpose(1, 0, 2).astype(np.float32)
    e0 = np.zeros((1, 9), np.float32)
    e0[0, 0] = 1.0
    return {
        "aq": conv_w(qd, qw2), "ak": conv_w(kd, kw2),
        "qcols": bias_cols(qb.astype(np.float64), qdb.astype(np.float64), qd),
        "kcols": bias_cols(kb.astype(np.float64), kdb.astype(np.float64), kd),
        "cv": cv, "pwT": pwT,
        "bv": bias_cols(vb.astype(np.float64), vdb.astype(np.float64), vd),
        "pbrow": pb.reshape(1, C).astype(np.float32),
        "e0row": e0,
        "ones1": np.ones((1, C), np.float32),
    }


def kernel(**inputs):
    if "nc" not in _CACHE:
        _CACHE["nc"] = _build_nc()
    nc = _CACHE["nc"]

    consts = _host_consts(**{k: np.asarray(inputs[k]) for k in
                             ("qw", "qb", "kw", "kb", "vw", "vb", "qdw", "qdb",
                              "kdw", "kdb", "vdw", "vdb", "pw", "pb")})
    x0 = np.asarray(inputs["x0"]).astype(BF_NP)
    x1 = np.asarray(inputs["x1"]).astype(BF_NP)
    x2 = np.asarray(inputs["x2"]).astype(BF_NP)
    n_cores = x0.shape[0]
    in_maps = [dict(consts, x0=x0[i], x1=x1[i], x2=x2[i]) for i in range(n_cores)]
    res = run_bass_kernel_spmd(nc, in_maps, list(range(n_cores)))
    _CACHE["last_res"] = res
    return np.stack([np.asarray(r["y"]) for r in res.results]).astype(np.float32)


def kernel_sim(**inputs):
    """CoreSim validation path: run sample 0 only through the simulator."""
    from concourse.bass_interp import CoreSim

    if "nc" not in _CACHE:
        _CACHE["nc"] = _build_nc()
    nc = _CACHE["nc"]
    consts = _host_consts(**{k: np.asarray(inputs[k]) for k in
                             ("qw", "qb", "kw", "kb", "vw", "vb", "qdw", "qdb",
                              "kdw", "kdb", "vdw", "vdb", "pw", "pb")})
    sim = CoreSim(nc)
    for name, arr in consts.items():
        sim.tensor(name)[:] = arr
    for name in ("x0", "x1", "x2"):
        sim.tensor(name)[:] = np.asarray(inputs[name])[0].astype(BF_NP)
    sim.simulate()
    return np.array(sim.tensor("y"))[None].astype(np.float32)
